# revision 48
# baseline (speedup 1.0000x reference)
"""Two-layer GCN (GCNConv -> ReLU -> GCNConv -> log_softmax) on 8 Trainium2
NeuronCores.

Strategy (graph/data parallel node partitioning):
  * Destination nodes are sharded across the 8 cores and packed into
    load-balanced 128-node "dst tiles" (host-side greedy balancing of
    per-(tile, src-core) edge counts across cores).
  * Phase 1: each core computes g = dinv * (x_shard @ W1) for its own nodes
    (fp16 matmuls, fp32 PSUM accumulate), stores its g-table shard in fp8
    (1024B rows -- dma_gather needs 256B-multiple rows).  The table is
    AllGathered in 4 quarter-chunks, each issued as soon as the local
    quarter is done, so comms overlap phase 1's tail and phase 3's head.
  * Phase 3: tiles are processed in pairs; per (pair, chunk) ONE
    `dma_gather` pulls the source rows for both tiles' in-edges (edges
    bucketed by table quarter so indices fit int16; trailing pad indices
    are dropped via num_idxs_reg).  A host-precomputed one-hot selection
    matrix S turns the per-destination segment-sum into fp8 DoubleRow PE
    matmuls (256 edges per matmul) accumulated in PSUM.  Tile epilogue:
    t1 = dinv*relu(dinv*acc + b1); g2 = t1 @ W2 via PE transposes; g2
    stored fp8 in 256B rows.
  * Phase 3.5: AllGather g2, in 4 quarter-chunks issued as phase 3
    crosses each quarter boundary.
  * Phase 4: same paired gather + S-matmul with S stationary (output
    lands [dst, feature]), then b2 and log_softmax (ACT-heavy, tails
    deferred one pair so gathers never wait on them).

Self-contained: hardcodes shapes; only needs the container toolchain at
/opt/trn_rl_repo.
"""

import os
import sys

for _p in ("/opt/trn_rl_repo",):
    if _p not in sys.path:
        sys.path.insert(0, _p)

import numpy as np
import ml_dtypes

import concourse.bacc as bacc
import concourse.bass as bass
import concourse.tile as tile
from concourse import bass_utils, mybir
from concourse.masks import make_identity

P = 128
FP16 = mybir.dt.float16
FP8 = mybir.dt.float8e4
F32 = mybir.dt.float32
I16 = mybir.dt.int16
I32 = mybir.dt.int32
AX = mybir.AxisListType
ALU = mybir.AluOpType
ACT = mybir.ActivationFunctionType
NP_FP8 = ml_dtypes.float8_e4m3


class Cfg:
    def __init__(self, n_nodes=100000, n_cores=8, f_in=1433, f_mid=789, f_out=7,
                 n_chunks=4, mm_free=512, group=2):
        self.n_nodes = n_nodes
        self.n_cores = n_cores
        self.f_in = f_in
        self.kc = (f_in + P - 1) // P          # k-chunks for layer-1 matmul
        self.f_mid = f_mid
        self.fmp = ((f_mid + 255) // 256) * 256   # fp8 row padded to 256B: 1024
        self.g2w = 256                         # fp8 g2 row: 256B
        self.kc2 = (f_mid + P - 1) // P        # k-chunks for layer-2 matmul
        self.f_out = f_out
        self.ns = n_nodes // n_cores           # nodes per shard (pre-pad)
        assert self.ns * n_cores == n_nodes
        self.t = (self.ns + P - 1) // P        # dst tiles per core
        self.nsp = self.t * P                  # padded shard size
        self.ntot = self.nsp * n_cores         # padded global table rows
        self.n_chunks = n_chunks               # int16 table chunks (quarters)
        # small LAST chunk: the serialized AllGather chain's tail gates the
        # in-order gather queue at (group0, last chunk), so keep it short
        last = max(1, self.t // 12)
        q = (self.t - last) // (n_chunks - 1)
        r = (self.t - last) - q * (n_chunks - 1)
        sizes = [q + (1 if i < r else 0) for i in range(n_chunks - 1)] + [last]
        self.qb = [0]
        for s in sizes:
            self.qb.append(self.qb[-1] + s)
        self.qrows = [s * P for s in sizes]           # local rows per quarter
        for s in sizes:
            assert s * P * n_cores < 32768
        self.mm_free = mm_free
        self.group = group                     # tiles per merged gather
        self.ngp = (self.t + group - 1) // group
        self.groups = [list(range(g * group, min((g + 1) * group, self.t)))
                       for g in range(self.ngp)]
        # set by preprocess:
        self.kb = None                         # [t][cb] blocks per bucket
        self.cntmax = None
        self.blk_off2 = None                   # [ngp][cb] merged-bucket offset
        self.sub_off = None                    # [t][cb] tile block offset
        self.btg = None                        # [ngp] blocks per group
        self.nblk_tot = None


# ----------------------------------------------------------------- host side

def _balance_tiles(indeg_cb, t_tiles):
    """Greedy: assign nodes to tiles (capacity 128) minimizing the max
    per-(tile, chunk) count.  indeg_cb: [n, NB] per-node in-edge counts."""
    n, NB = indeg_cb.shape
    tot = indeg_cb.sum(axis=1)
    order = np.argsort(-tot, kind="stable")
    cnt = np.zeros((t_tiles, NB), dtype=np.int64)
    fill = np.zeros(t_tiles, dtype=np.int64)
    assign = np.zeros(n, dtype=np.int64)
    cols = np.zeros(n, dtype=np.int64)
    for d in order:
        v = indeg_cb[d]
        cand = cnt + v[None, :]
        score = cand.max(axis=1) + 1e-3 * cand.sum(axis=1)
        score[fill >= P] = 1 << 60
        t = int(np.argmin(score))
        assign[d] = t
        cols[d] = fill[t]
        fill[t] += 1
        cnt[t] += v
    return assign, cols


def preprocess(x, edge_index, W1, b1, W2, b2, cfg):
    """Shard + permute nodes, bucket edges by (dst tile, src quarter)."""
    N, C = cfg.n_nodes, cfg.n_cores
    src = np.asarray(edge_index[0], dtype=np.int64)
    dst = np.asarray(edge_index[1], dtype=np.int64)
    loop = np.arange(N, dtype=np.int64)
    src = np.concatenate([src, loop])
    dst = np.concatenate([dst, loop])

    deg = np.bincount(dst, minlength=N).astype(np.float64)
    dinv = (1.0 / np.sqrt(deg)).astype(np.float32)

    NB = cfg.n_chunks
    qb_r = np.array([cfg.qb[j] * P for j in range(NB + 1)])  # local-row bounds
    shard_of = dst // cfg.ns
    shard_src = src // cfg.ns

    # A node's src-quarter depends on its owner's tile assignment, which is
    # what we are choosing -- so balance on per-source-core counts instead
    # (quarter splits within a core come out near-uniform by randomness).
    indeg_src = np.zeros((N, C), dtype=np.int64)
    np.add.at(indeg_src, (dst, shard_src), 1)

    node_tile = np.zeros(N, dtype=np.int64)
    node_col = np.zeros(N, dtype=np.int64)
    pg = np.zeros(N, dtype=np.int64)
    nodes_of = []
    for c in range(C):
        ids = np.arange(c * cfg.ns, (c + 1) * cfg.ns)
        assign, cols = _balance_tiles(indeg_src[ids], cfg.t)
        node_tile[ids] = assign
        node_col[ids] = cols
        pg[ids] = c * cfg.nsp + assign * P + cols
        nv = np.full(cfg.nsp, -1, dtype=np.int64)
        nv[assign * P + cols] = ids
        nodes_of.append(nv)

    # ---- bucket edges by (core, dst tile, src quarter)
    e_tile = node_tile[dst]
    loc_src = pg[src] - shard_src * cfg.nsp           # row within owner shard
    e_chunk = np.searchsorted(qb_r, loc_src, side="right") - 1
    qsz = np.array(cfg.qrows)[e_chunk]
    e_idx = shard_src * qsz + (loc_src - qb_r[e_chunk])
    e_dcol = node_col[dst]

    counts = np.zeros((C, cfg.t, NB), dtype=np.int64)
    np.add.at(counts, (shard_of, e_tile, e_chunk), 1)
    kb = ((counts.max(axis=0) + P - 1) // P).astype(np.int64)   # [t, NB]
    kb = np.maximum(kb, 1)
    cfg.kb = kb
    cfg.cntmax = counts.max(axis=0)           # [t, NB]

    # merged (group, chunk) block layout
    ngp = cfg.ngp
    blk_off2 = np.zeros((ngp, NB), dtype=np.int64)
    sub_off = np.zeros((cfg.t, NB), dtype=np.int64)
    run = 0
    for gp in range(ngp):
        for cb in range(NB):
            blk_off2[gp, cb] = run
            for t in cfg.groups[gp]:
                sub_off[t, cb] = run
                run += kb[t, cb]
    cfg.blk_off2 = blk_off2
    cfg.sub_off = sub_off
    cfg.btg = np.array([sum(int(kb[t, cb]) for t in cfg.groups[gp]
                            for cb in range(NB)) for gp in range(ngp)])
    nblk_tot = int(run)
    cfg.nblk_tot = nblk_tot

    order_all = np.lexsort((e_chunk, e_tile, shard_of))
    s_sorted = e_idx[order_all].astype(np.int16)
    d_sorted = e_dcol[order_all].astype(np.int64)
    key = (shard_of * cfg.t + e_tile)[order_all] * NB + e_chunk[order_all]
    bounds = np.searchsorted(key, np.arange(C * cfg.t * NB + 1))

    xpad = np.zeros((cfg.kc * P, N), dtype=np.float16)
    xpad[: cfg.f_in, :] = np.asarray(x, dtype=np.float32).T.astype(np.float16)
    w1h = np.zeros((P, cfg.kc, cfg.f_mid), dtype=np.float16)
    w1t = np.zeros((cfg.kc * P, cfg.f_mid), dtype=np.float32)
    w1t[: cfg.f_in] = np.asarray(W1, dtype=np.float32)
    w1h[:] = w1t.reshape(cfg.kc, P, cfg.f_mid).transpose(1, 0, 2).astype(np.float16)
    w2h = np.zeros((P, cfg.kc2, cfg.f_out), dtype=np.float16)
    w2t = np.zeros((cfg.kc2 * P, cfg.f_out), dtype=np.float32)
    w2t[: cfg.f_mid] = np.asarray(W2, dtype=np.float32)
    w2h[:] = w2t.reshape(cfg.kc2, P, cfg.f_out).transpose(1, 0, 2).astype(np.float16)
    b1r = np.tile(np.asarray(b1, dtype=np.float32)[None, :], (P, 1))
    b2r = np.zeros((P, 8), dtype=np.float32)
    b2r[:, : cfg.f_out] = np.asarray(b2, dtype=np.float32)[None, :]

    in_maps = []
    for c in range(C):
        nv = nodes_of[c]
        valid = nv >= 0
        xs = np.zeros((cfg.kc * P, cfg.nsp), dtype=np.float16)
        xs[:, valid] = xpad[:, nv[valid]]
        # [P, T, KC, P]: tile-major so each tile's load is contiguous per line
        xt = np.ascontiguousarray(
            xs.reshape(cfg.kc, P, cfg.t, P).transpose(1, 2, 0, 3))
        dvt = np.zeros(cfg.nsp, dtype=np.float32)
        dvt[valid] = dinv[nv[valid]]
        dv = np.ascontiguousarray(dvt.reshape(cfg.t, P).T)
        # idx: per block range: int16, idx j at [j%16, off*8 + j//16]
        eidx = np.zeros((P, nblk_tot * 8), dtype=np.int16)
        ss_p = np.zeros(nblk_tot * P, dtype=np.int64)
        ss_b = np.zeros(nblk_tot * P, dtype=np.int64)
        ss_c = np.zeros(nblk_tot * P, dtype=np.int64)
        nss = 0
        for t in range(cfg.t):
            for cb in range(NB):
                lo = bounds[(c * cfg.t + t) * NB + cb]
                hi = bounds[(c * cfg.t + t) * NB + cb + 1]
                cnt = hi - lo
                nsl = int(kb[t, cb]) * P
                off = int(sub_off[t, cb])
                # pad with index 0 (harmless extra gathers; S rows are zero)
                ai = np.zeros(nsl, dtype=np.int16)
                ai[:cnt] = s_sorted[lo:hi]
                eidx[:, off * 8: off * 8 + nsl // 16] = np.tile(
                    ai.reshape(nsl // 16, 16).T, (8, 1))
                j = np.arange(cnt)
                ss_p[nss:nss + cnt] = j % P
                ss_b[nss:nss + cnt] = off + j // P
                ss_c[nss:nss + cnt] = d_sorted[lo:hi]
                nss += cnt
        ss = np.zeros((P, nblk_tot, P), dtype=NP_FP8)
        ss[ss_p[:nss], ss_b[:nss], ss_c[:nss]] = NP_FP8(1.0)
        in_maps.append({
            "xt": xt, "w1": w1h, "w2": w2h, "b1r": b1r, "b2r": b2r,
            "dinv_t": dv, "eidx": eidx, "ss": ss,
        })
    return in_maps, nodes_of


# --------------------------------------------------------------- device side

def build(cfg, debug=False):
    nc = bacc.Bacc("TRN2", target_bir_lowering=False, debug=debug,
                   enable_asserts=False, num_devices=cfg.n_cores,
                   num_swdge_queues=4)
    T, NB = cfg.t, cfg.n_chunks
    FM, FMP, FO, KC, KC2 = cfg.f_mid, cfg.fmp, cfg.f_out, cfg.kc, cfg.kc2
    G2W = cfg.g2w
    kb, cntmax = cfg.kb, cfg.cntmax
    blk_off2, sub_off, btg = cfg.blk_off2, cfg.sub_off, cfg.btg
    nblk_tot = cfg.nblk_tot
    ngp, groups = cfg.ngp, cfg.groups
    btgmax = int(btg.max())
    kbgmax = max(int(sum(kb[t, cb] for t in groups[gp]))
                 for gp in range(ngp) for cb in range(NB))
    qb, qrows = cfg.qb, cfg.qrows
    use_dr = not os.environ.get("GCN_NO_DR")

    def nreg16(gp, cb):
        """Merged-bucket index count with the last tile's pad tail dropped."""
        tl = groups[gp][-1]
        head = int(sub_off[tl, cb] - blk_off2[gp, cb]) * P
        tailc = min(int(kb[tl, cb]) * P,
                    max(16, -(-int(cntmax[tl, cb]) // 16) * 16))
        return head + tailc

    xt_d = nc.dram_tensor("xt", [P, T, KC, P], FP16, kind="ExternalInput").ap()
    w1_d = nc.dram_tensor("w1", [P, KC, FM], FP16, kind="ExternalInput").ap()
    w2_d = nc.dram_tensor("w2", [P, KC2, FO], FP16, kind="ExternalInput").ap()
    b1_d = nc.dram_tensor("b1r", [P, FM], F32, kind="ExternalInput").ap()
    b2_d = nc.dram_tensor("b2r", [P, 8], F32, kind="ExternalInput").ap()
    dv_d = nc.dram_tensor("dinv_t", [P, T], F32, kind="ExternalInput").ap()
    ei_d = nc.dram_tensor("eidx", [P, nblk_tot * 8], I16, kind="ExternalInput").ap()
    ss_d = nc.dram_tensor("ss", [P, nblk_tot, P], FP8, kind="ExternalInput").ap()
    out_d = nc.dram_tensor("out", [cfg.nsp, FO], F32, kind="ExternalOutput").ap()

    rg = [list(range(cfg.n_cores))]

    with tile.TileContext(nc) as tc:
        with tc.tile_pool(name="res", bufs=1) as res, \
             tc.tile_pool(name="dram", bufs=1, space="DRAM") as dram:
            g_local = dram.tile([cfg.nsp, FMP], FP8)
            g_chunk = [dram.tile([cfg.n_cores * qrows[j], FMP], FP8,
                                 addr_space="Shared", name=f"g_chunk{j}")
                       for j in range(NB)]
            g2_local = dram.tile([cfg.nsp, G2W], FP8)
            g2_chunk = [dram.tile([cfg.n_cores * qrows[j], G2W], FP8,
                                  addr_space="Shared", name=f"g2_chunk{j}")
                        for j in range(NB)]

            w2_sb = res.tile([P, KC2, FO], FP16)
            nc.sync.dma_start(out=w2_sb[:], in_=w2_d[:])
            b1_sb = res.tile([P, FM], F32)
            nc.sync.dma_start(out=b1_sb[:], in_=b1_d[:])
            b2_sb = res.tile([P, 8], F32)
            nc.sync.dma_start(out=b2_sb[:], in_=b2_d[:])
            dv_sb = res.tile([P, T], F32)
            nc.sync.dma_start(out=dv_sb[:], in_=dv_d[:])
            ident = res.tile([P, P], F32)
            make_identity(nc, ident[:])
            ident_h = res.tile([P, P], FP16)
            nc.vector.tensor_copy(out=ident_h[:], in_=ident[:])

            # ---------------- phase 1: g = dinv * (x @ W1)  (fp8 table),
            # with quarter-AllGathers issued as quarters complete
            qnext = 0
            with tc.tile_pool(name="p1", bufs=3) as p1, \
                 tc.tile_pool(name="p1w", bufs=1) as p1w, \
                 tc.tile_pool(name="p1ps", bufs=2, space="PSUM") as p1ps:
                w1_sb = p1w.tile([P, KC, FM], FP16)
                nc.sync.dma_start(out=w1_sb[:], in_=w1_d[:])
                for t in range(T):
                    xtile = p1.tile([P, KC, P], FP16, tag="xtile")
                    nc.sync.dma_start(out=xtile[:], in_=xt_d[:, t])
                    hp = p1ps.tile([P, FM], F32, tag="hp")
                    for f0 in range(0, FM, cfg.mm_free):
                        f1 = min(f0 + cfg.mm_free, FM)
                        for c in range(KC):
                            nc.tensor.matmul(
                                out=hp[:, f0:f1], lhsT=xtile[:, c, :],
                                rhs=w1_sb[:, c, f0:f1],
                                start=(c == 0), stop=(c == KC - 1))
                    gt = p1.tile([P, FMP], FP8, tag="gt")
                    nc.vector.memset(gt[:, FM:], 0.0)
                    nc.scalar.activation(
                        out=gt[:, :FM], in_=hp[:], func=ACT.Copy,
                        scale=dv_sb[:, t:t + 1])
                    nc.sync.dma_start(out=g_local[t * P:(t + 1) * P, :], in_=gt[:])
                    if t == qb[qnext + 1] - 1:
                        nc.gpsimd.collective_compute(
                            "AllGather", ALU.bypass, replica_groups=rg,
                            ins=[g_local[qb[qnext] * P: qb[qnext + 1] * P, :]],
                            outs=[g_chunk[qnext][:]])
                        qnext += 1

            # ---------------- phase 3: paired gathers, inline epilogues
            with tc.tile_pool(name="p3", bufs=3) as p3, \
                 tc.tile_pool(name="p3e", bufs=2) as p3e, \
                 tc.tile_pool(name="p3g", bufs=4) as p3g, \
                 tc.tile_pool(name="p3acc", bufs=1, space="PSUM") as p3acc, \
                 tc.tile_pool(name="p3ps", bufs=1, space="PSUM") as p3ps:
                q2next = 0

                def p3_epilogue(t, acc):
                    nonlocal q2next
                    # t1 = dinv*acc (ACT) ; += b1 ; t1h = dinv*relu(t1) fp16
                    t1 = p3e.tile([P, FM], F32, tag="t1")
                    nc.scalar.activation(
                        out=t1[:], in_=acc[:], func=ACT.Copy,
                        scale=dv_sb[:, t:t + 1])
                    nc.vector.tensor_add(out=t1[:], in0=t1[:], in1=b1_sb[:])
                    # dinv > 0, so dinv*relu(t1) == relu(dinv*t1): one ACT op
                    t1h = p3e.tile([P, FM], FP16, tag="t1h")
                    nc.scalar.activation(
                        out=t1h[:], in_=t1[:], func=ACT.Relu,
                        scale=dv_sb[:, t:t + 1])
                    # g2T = W2^T @ t1h^T  -> [FO, 128]
                    g2t = p3ps.tile([P, P], F32, tag="g2t")
                    for c in range(KC2):
                        f0 = c * P
                        cw = min(P, FM - f0)
                        tp = p3ps.tile([P, P], FP16, tag="tp")
                        nc.tensor.transpose(
                            out=tp[:cw, :], in_=t1h[:, f0:f0 + cw],
                            identity=ident_h[:])
                        tps = p3e.tile([P, P], FP16, tag="tps")
                        nc.vector.tensor_copy(out=tps[:cw, :], in_=tp[:cw, :])
                        nc.tensor.matmul(
                            out=g2t[:FO, :], lhsT=w2_sb[:cw, c, :], rhs=tps[:cw, :],
                            start=(c == 0), stop=(c == KC2 - 1))
                    g2s = p3e.tile([P, P], FP16, tag="g2s")
                    nc.vector.tensor_copy(out=g2s[:FO, :], in_=g2t[:FO, :])
                    g2ntp = p3ps.tile([P, P], FP16, tag="tp")
                    nc.tensor.transpose(
                        out=g2ntp[:, :FO], in_=g2s[:FO, :],
                        identity=ident_h[:FO, :FO])
                    g2o = p3e.tile([P, G2W], FP8, tag="g2o")
                    nc.vector.memset(g2o[:], 0.0)
                    nc.vector.tensor_copy(out=g2o[:, :FO], in_=g2ntp[:, :FO])
                    nc.sync.dma_start(
                        out=g2_local[t * P:(t + 1) * P, :], in_=g2o[:])
                    if t == qb[q2next + 1] - 1:
                        nc.gpsimd.collective_compute(
                            "AllGather", ALU.bypass, replica_groups=rg,
                            ins=[g2_local[qb[q2next] * P: qb[q2next + 1] * P, :]],
                            outs=[g2_chunk[q2next][:]])
                        q2next += 1

                def emit_blocks(acc, sst, gg, lb, gb, kbb, start0, stop0):
                    """Matmuls for one tile's kbb blocks: sst blocks at lb..,
                    gg blocks at gb..; start/stop flags for the tile group."""
                    if use_dr:
                        for b in range(0, kbb - 1, 2):
                            for f0 in range(0, FM, cfg.mm_free):
                                f1 = min(f0 + cfg.mm_free, FM)
                                nc.tensor.matmul(
                                    out=acc[:, f0:f1],
                                    lhsT=sst[:, lb + b: lb + b + 2, :],
                                    rhs=gg[:, gb + b: gb + b + 2, f0:f1],
                                    perf_mode=mybir.MatmulPerfMode.DoubleRow,
                                    start=(start0 and b == 0),
                                    stop=(stop0 and b + 2 >= kbb))
                        if kbb % 2:
                            b = kbb - 1
                            for f0 in range(0, FM, cfg.mm_free):
                                f1 = min(f0 + cfg.mm_free, FM)
                                nc.tensor.matmul(
                                    out=acc[:, f0:f1], lhsT=sst[:, lb + b, :],
                                    rhs=gg[:, gb + b, f0:f1],
                                    start=(start0 and b == 0),
                                    stop=stop0)
                    else:
                        for b in range(kbb):
                            for f0 in range(0, FM, cfg.mm_free):
                                f1 = min(f0 + cfg.mm_free, FM)
                                nc.tensor.matmul(
                                    out=acc[:, f0:f1], lhsT=sst[:, lb + b, :],
                                    rhs=gg[:, gb + b, f0:f1],
                                    start=(start0 and b == 0),
                                    stop=(stop0 and b == kbb - 1))

                for gp in range(ngp):
                    g0 = int(blk_off2[gp, 0])
                    btt = int(btg[gp])
                    eit = p3.tile([P, btgmax * 8], I16, tag="eit")
                    nc.sync.dma_start(
                        out=eit[:, : btt * 8],
                        in_=ei_d[:, g0 * 8: (g0 + btt) * 8])
                    sst = p3.tile([P, btgmax, P], FP8, tag="sst")
                    nc.sync.dma_start(
                        out=sst[:, :btt, :], in_=ss_d[:, g0: g0 + btt, :])
                    accs = {t: p3acc.tile([P, FM], F32, tag=f"acc{i}",
                                          name=f"acc{i}")
                            for i, t in enumerate(groups[gp])}
                    for cb in range(NB):
                        kbg = int(sum(kb[t, cb] for t in groups[gp]))
                        ni = kbg * P
                        co = int(blk_off2[gp, cb]) - g0
                        if gp == 0:
                            gg = p3g.tile([P, kbgmax, FMP], FP8, tag="gg")
                            nc.vector.memset(gg[:], 0.0)
                            gv = gg[:, :kbg, :]
                        else:
                            gg = p3g.tile([P, kbg, FMP], FP8, tag="gg",
                                          padded_shape=[P, kbgmax, FMP])
                            gv = gg[:, :, :]
                        nc.gpsimd.dma_gather(
                            out_ap=gv,
                            in_ap=g_chunk[cb][:],
                            idxs_ap=eit[:, co * 8: co * 8 + ni // 16],
                            num_idxs=ni, num_idxs_reg=nreg16(gp, cb),
                            elem_size=FMP, single_packet=(ni <= 1024),
                            queue_num=(gp * NB + cb) % 4)
                        for t in groups[gp]:
                            lb = int(sub_off[t, cb]) - g0
                            gb = int(sub_off[t, cb]) - int(blk_off2[gp, cb])
                            emit_blocks(accs[t], sst, gg, lb, gb,
                                        int(kb[t, cb]),
                                        start0=(cb == 0), stop0=(cb == NB - 1))
                    for t in groups[gp]:
                        p3_epilogue(t, accs[t])

            # ---------------- phase 4: paired gathers, deferred tails
            with tc.tile_pool(name="p4", bufs=3) as p4, \
                 tc.tile_pool(name="p4e", bufs=3) as p4e, \
                 tc.tile_pool(name="p4g", bufs=6) as p4g, \
                 tc.tile_pool(name="p4ps", bufs=4, space="PSUM") as p4ps:

                def p4_tail(t, acc2):
                    tf = p4e.tile([P, 8], F32, tag="tf")
                    nc.scalar.activation(
                        out=tf[:], in_=acc2[:], func=ACT.Copy,
                        scale=dv_sb[:, t:t + 1])
                    nc.vector.tensor_add(out=tf[:], in0=tf[:], in1=b2_sb[:])
                    nm = p4e.tile([P, 1], F32, tag="nm")
                    nc.vector.tensor_reduce(
                        out=nm[:], in_=tf[:, :FO], axis=AX.X, op=ALU.max,
                        negate=True)
                    ex = p4e.tile([P, 8], F32, tag="ex")
                    se = p4e.tile([P, 1], F32, tag="se")
                    nc.scalar.activation(
                        out=ex[:, :FO], in_=tf[:, :FO], func=ACT.Exp,
                        bias=nm[:, :1], scale=1.0, accum_out=se[:, :1])
                    lse = p4e.tile([P, 1], F32, tag="lse")
                    nc.scalar.activation(out=lse[:], in_=se[:], func=ACT.Ln)
                    bias2 = p4e.tile([P, 1], F32, tag="bias2")
                    nc.vector.tensor_tensor(
                        out=bias2[:], in0=nm[:], in1=lse[:], op=ALU.subtract)
                    of = p4e.tile([P, 8], F32, tag="of")
                    nc.scalar.activation(
                        out=of[:, :FO], in_=tf[:, :FO], func=ACT.Identity,
                        bias=bias2[:, :1])
                    nc.sync.dma_start(out=out_d[t * P:(t + 1) * P, :],
                                      in_=of[:, :FO])

                prev = []
                for gp in range(ngp):
                    g0 = int(blk_off2[gp, 0])
                    btt = int(btg[gp])
                    eit = p4.tile([P, btgmax * 8], I16, tag="eit4")
                    nc.sync.dma_start(
                        out=eit[:, : btt * 8],
                        in_=ei_d[:, g0 * 8: (g0 + btt) * 8])
                    sst = p4.tile([P, btgmax, P], FP8, tag="sst4")
                    nc.sync.dma_start(
                        out=sst[:, :btt, :], in_=ss_d[:, g0: g0 + btt, :])
                    acc2s = {t: p4ps.tile([P, 8], F32, tag=f"acc2_{i}",
                                          name=f"acc2_{i}")
                             for i, t in enumerate(groups[gp])}
                    for cb in range(NB):
                        kbg = int(sum(kb[t, cb] for t in groups[gp]))
                        ni = kbg * P
                        co = int(blk_off2[gp, cb]) - g0
                        if gp == 0:
                            gg2 = p4g.tile([P, kbgmax, G2W], FP8, tag="gg2")
                            nc.vector.memset(gg2[:], 0.0)
                            gv2 = gg2[:, :kbg, :]
                        else:
                            gg2 = p4g.tile([P, kbg, G2W], FP8, tag="gg2",
                                           padded_shape=[P, kbgmax, G2W])
                            gv2 = gg2[:, :, :]
                        nc.gpsimd.dma_gather(
                            out_ap=gv2,
                            in_ap=g2_chunk[cb][:],
                            idxs_ap=eit[:, co * 8: co * 8 + ni // 16],
                            num_idxs=ni, num_idxs_reg=nreg16(gp, cb),
                            elem_size=G2W, single_packet=(ni <= 1024),
                            queue_num=(gp * NB + cb) % 4)
                        for t in groups[gp]:
                            lb = int(sub_off[t, cb]) - g0
                            gb = int(sub_off[t, cb]) - int(blk_off2[gp, cb])
                            kbb = int(kb[t, cb])
                            for b in range(kbb):
                                nc.tensor.matmul(
                                    out=acc2s[t][:, :8],
                                    lhsT=sst[:, lb + b, :],
                                    rhs=gg2[:, gb + b, :8],
                                    start=(cb == 0 and b == 0),
                                    stop=(cb == NB - 1 and b == kbb - 1))
                    for t, a2 in prev:
                        p4_tail(t, a2)
                    prev = list(acc2s.items())
                for t, a2 in prev:
                    p4_tail(t, a2)

    nc.compile()
    return nc


# ------------------------------------------------------------------ runner

def _run(inputs, cfg=None, trace=False, trace_kwargs=None):
    cfg = cfg or Cfg()
    in_maps, nodes_of = preprocess(
        inputs["x"], inputs["edge_index"], inputs["W1"], inputs["b1"],
        inputs["W2"], inputs["b2"], cfg)
    nc = build(cfg)
    res = bass_utils.run_bass_kernel_spmd(
        nc, in_maps, core_ids=list(range(cfg.n_cores)), trace=trace,
        **(trace_kwargs or {}))
    out = np.zeros((cfg.n_nodes, cfg.f_out), dtype=np.float32)
    for c in range(cfg.n_cores):
        oc = res.results[c]["out"]
        nv = nodes_of[c]
        valid = nv >= 0
        out[nv[valid]] = oc[valid]
    return out, res


def kernel(**inputs):
    out, _ = _run(inputs)
    return out
